# revision 32
# baseline (speedup 1.0000x reference)
"""Trainium2 Bass kernel for nn_LCN (locally-connected network) — v2.

Computation:
  x: (512, 1, 280, 280) -> non-overlapping 28x28 patches (10x10 grid, P=100)
  y[b, f, p] = sum_{k,l} x[b, 28ph+k, 28pw+l] * w[f*100+p, 0, k, l]
  y = relu(y + bias[f*100+p]);  out = y_flat @ dec_w.T + dec_b  (j = f*100+p)

Sharding: 8 cores = 2 batch halves (256 imgs) x 4 patch quarters (25 patches).
x is pre-transposed and cast to bf16 on the host into [pt=175, dd=112, b=256]
blocks (pt = patch*7 + chunk, dd = pixel-in-chunk), so the device does ZERO
transposes and half the DMA bytes vs fp32.

Per core pipeline (all matmuls bf16, 1 cycle/row vs fp32's 4):
  - 7 group DMAs (4 patches each; last group 1 patch), ~1.6 MB apiece,
    rearranged in-flight to SBUF [112, (pt b)]
  - per group: 28 accumulating matmuls lhsT=w[112,16], rhs=x[112,256], four
    patches packed per PSUM bank at partition offsets 0/32/64/96
  - ACT: relu(y + bias) -> y_sb bf16
  - decoder matmul accumulates dec[128,10]^T @ y[128,256] -> out_ps [10,256]
Host combines: out[b] = sum over 4 patch-quarters of partial[o,b] + dec_b.
"""

import sys

import numpy as np

for _p in ("/opt/trn_rl_repo", "/opt/trn_rl_repo/concourse"):
    if _p not in sys.path:
        sys.path.insert(0, _p)

import concourse.bass as bass
import concourse.mybir as mybir
import concourse.tile as tile
from concourse import bacc

F32 = mybir.dt.float32
BF16 = mybir.dt.bfloat16

# Problem constants
B, H, W = 512, 280, 280
KS = 28
HS = WS = 10
P = 100
F = 16
OUT = 10
NCORES = 8
NB = 256        # images per core (batch half)
NP = 25         # patches per core (patch quarter)
NT = 7          # 112-pixel chunks per patch
CK = 112        # contraction chunk (4 rows x 28 cols)
NG = 7          # 4-patch groups per core (6 full + 1 single)
XCOLS = NP * NT * NB        # 44800 columns: (p, t, b), b innermost


def build_program():
    nc = bacc.Bacc("TRN2")
    x_d = nc.dram_tensor("x", [NG, CK * 28 * NB], BF16, kind="ExternalInput")
    w_d = nc.dram_tensor("w", [CK, NP * NT * F], BF16, kind="ExternalInput")
    b_d = nc.dram_tensor("bias", [128, NG], F32, kind="ExternalInput")
    d_d = nc.dram_tensor("dec", [128, NG * OUT], BF16, kind="ExternalInput")
    o_d = nc.dram_tensor("out", [OUT, NB], F32, kind="ExternalOutput")

    with tile.TileContext(nc) as tc:
        with (
            tc.tile_pool(name="const", bufs=1) as constp,
            tc.tile_pool(name="xg", bufs=7) as xgp,
            tc.tile_pool(name="yps", bufs=4, space="PSUM") as ypsp,
            tc.tile_pool(name="ops", bufs=1, space="PSUM") as opsp,
        ):
            zero_sb = constp.tile([128, NB], F32)
            nc.vector.memset(zero_sb[:], 0.0)
            bias_sb = constp.tile([128, NG], F32)
            dec_sb = constp.tile([128, NG * OUT], BF16)
            w_sb = constp.tile([CK, NP * NT * F], BF16)
            y_sb = constp.tile([128, NG * NB], BF16)

            # x group DMAs up front, each group split across BOTH HWDGE
            # rings (sync + scalar) at matching positions so both halves
            # drain concurrently and groups still complete in order.
            # Group 0's halves dispatch FIRST (stream starts ~1.2us sooner);
            # the small consts follow right behind.
            xg_tiles = []
            for g in range(NG):
                npg = 4 if g < NG - 1 else NP - 4 * (NG - 1)
                t_ = xgp.tile([CK, 28 * NB], BF16, name="xg")
                cols = npg * NT * NB
                hcols = (2 * NT * NB) if npg == 4 else cols
                src = x_d[g:g + 1, :CK * cols].rearrange(
                    "a (dd c) -> (a dd) c", dd=CK)
                nc.sync.dma_start(out=t_[:, :hcols], in_=src[:, :hcols])
                if hcols < cols:
                    nc.scalar.dma_start(
                        out=t_[:, hcols:cols], in_=src[:, hcols:])
                xg_tiles.append(t_)
                if g == 0:
                    nc.sync.dma_start(out=bias_sb[:], in_=b_d[:])
                    nc.sync.dma_start(out=dec_sb[:], in_=d_d[:])
                    nc.scalar.dma_start(out=w_sb[:], in_=w_d[:])

            def dec_mm(g):
                nc.tensor.matmul(
                    out_ps[:OUT, :NB],
                    dec_sb[:, g * OUT:(g + 1) * OUT],
                    y_sb[:, g * NB:(g + 1) * NB],
                    start=(g == 0),
                    stop=(g == NG - 1),
                )

            out_ps = opsp.tile([128, 512], F32)
            for g in range(NG):
                npg = 4 if g < NG - 1 else NP - 4 * (NG - 1)
                yt = ypsp.tile([128, 512], F32, name="y_ps")
                # make gap partitions (32q+16..32q+32) finite for the ACT read
                nc.vector.tensor_copy(yt[:, :NB], zero_sb[:])
                xg = xg_tiles[g]
                for q in range(npg):
                    pl = 4 * g + q
                    for t in range(NT):
                        nc.tensor.matmul(
                            yt[32 * q:32 * q + F, :NB],
                            w_sb[:, (pl * NT + t) * F:(pl * NT + t + 1) * F],
                            xg[:, (q * NT + t) * NB:(q * NT + t + 1) * NB],
                            start=(t == 0),
                            stop=(t == NT - 1),
                            tile_position=(0, 32 * q),
                        )
                nc.scalar.activation(
                    out=y_sb[:, g * NB:(g + 1) * NB],
                    in_=yt[:, :NB],
                    func=mybir.ActivationFunctionType.Relu,
                    bias=bias_sb[:, g:g + 1],
                )
                # decoder matmul lags one group so its wait on ACT(g-1)
                # never blocks group g's matmuls in the strict-FIFO PE queue
                if g > 0:
                    dec_mm(g - 1)
            dec_mm(NG - 1)

            out_sb = constp.tile([OUT, NB], F32)
            nc.vector.tensor_copy(out_sb[:], out_ps[:OUT, :NB])
            # two half-width output DMAs on separate rings: completion
            # receipts overlap instead of serializing at the very end
            nc.sync.dma_start(out=o_d[:, :NB // 2], in_=out_sb[:, :NB // 2])
            nc.scalar.dma_start(out=o_d[:, NB // 2:], in_=out_sb[:, NB // 2:])

    return nc


def stage_quarter(weight, bias, dec_w, qc):
    """Stage weights/bias/decoder for patch quarter qc (patches 25qc..25qc+24)."""
    import ml_dtypes

    weight = np.asarray(weight, np.float32)
    bias = np.asarray(bias, np.float32)
    dec_w = np.asarray(dec_w, np.float32)

    # w: (1600,1,28,28) -> (f, ph, pw, t, k4, l) -> [dd=(k4 l), (p t f)]
    w6 = weight.reshape(F, HS, WS, NT, 4, KS).transpose(4, 5, 1, 2, 3, 0)
    wt = np.ascontiguousarray(
        w6.reshape(CK, P, NT, F)[:, 25 * qc:25 * qc + NP])
    wst = wt.reshape(CK, NP * NT * F).astype(ml_dtypes.bfloat16)

    b5 = bias.reshape(F, P)
    d5 = dec_w.reshape(OUT, F, P)
    bst = np.zeros((128, NG), np.float32)
    dst = np.zeros((128, NG * OUT), np.float32)
    for pl in range(NP):
        g, q = divmod(pl, 4)
        p = 25 * qc + pl
        bst[32 * q:32 * q + F, g] = b5[:, p]
        dst[32 * q:32 * q + F, g * OUT:(g + 1) * OUT] = d5[:, :, p].T
    return wst, bst, dst.astype(ml_dtypes.bfloat16)


def stage_x(x):
    """x (512,1,280,280) f32 -> global [dd=112, p, t, b] bf16 (u16 view).

    Two cache-friendly steps: permute to (b, p, t, dd) with the 112-byte
    l-runs contiguous, then 700 small L2-resident [512,112]->[112,512]
    block transposes to get dd-major.
    """
    import ml_dtypes

    xr = np.asarray(x, np.float32).reshape(B, HS, NT, 4, WS, KS)
    # (b, ph, t, k4, pw, l) -> (b, ph, pw, t, k4, l)
    y1 = np.ascontiguousarray(xr.transpose(0, 1, 4, 2, 3, 5))
    y1 = y1.reshape(B, P * NT, CK).astype(ml_dtypes.bfloat16).view(np.uint16)
    g = np.empty((CK, P * NT, B), np.uint16)
    for pt in range(P * NT):
        g[:, pt, :] = y1[:, pt, :].T
    return g.reshape(CK, P, NT, B)


_cache = {}


def _get_nc():
    if "nc" not in _cache:
        nc = build_program()
        nc.finalize()
        _cache["nc"] = nc
    return _cache["nc"]


def make_in_maps(x, weight, bias, dec_w):
    import ml_dtypes

    xh = stage_x(x)  # (112, 100, 7, 512) u16 (bf16 bits)
    quarters = [stage_quarter(weight, bias, dec_w, qc) for qc in range(4)]
    in_maps = []
    for core in range(NCORES):
        h, qc = divmod(core, 4)
        xs = np.ascontiguousarray(
            xh[:, 25 * qc:25 * qc + NP, :, NB * h:NB * h + NB]
        ).reshape(CK, XCOLS)
        # chunk-contiguous layout: row g = group g's [dd, cols] block flat
        xr = np.zeros((NG, CK * 28 * NB), np.uint16)
        for g in range(NG):
            npg = 4 if g < NG - 1 else NP - 4 * (NG - 1)
            cols = npg * NT * NB
            c0 = g * 28 * NB
            xr[g, :CK * cols] = xs[:, c0:c0 + cols].reshape(-1)
        wst, bst, dst = quarters[qc]
        in_maps.append({"x": xr.view(ml_dtypes.bfloat16),
                        "w": wst, "bias": bst, "dec": dst})
    return in_maps


def combine(results, dec_b):
    out = np.zeros((B, OUT), np.float32)
    dec_b = np.asarray(dec_b, np.float32)
    for h in range(2):
        acc = np.zeros((OUT, NB), np.float32)
        for qc in range(4):
            acc += results[h * 4 + qc]["out"]
        out[NB * h:NB * h + NB] = acc.T + dec_b
    return out


def _install_ntff_hook():
    """Provide the missing antenv.axon_hooks module so trace=True works
    under axon (replicates trn_boot._ntff_profile_via_ctypes)."""
    import contextlib
    import ctypes
    import types

    if "antenv.axon_hooks" in sys.modules:
        return
    so_path = "/opt/axon/libaxon_pjrt.so"
    holder = {}
    mod = types.ModuleType("antenv.axon_hooks")
    mod.set_axon_ntff_profile_hook = lambda h: holder.__setitem__("h", h)
    mod.get_axon_ntff_profile_hook = lambda: holder.get("h")
    sys.modules["antenv.axon_hooks"] = mod
    try:
        import antenv
        antenv.axon_hooks = mod
    except ImportError:
        pass

    lib = ctypes.CDLL(so_path)
    if not hasattr(lib, "axon_start_nrt_profile"):
        return
    lib.axon_start_nrt_profile.argtypes = [
        ctypes.POINTER(ctypes.c_int64), ctypes.c_size_t]
    lib.axon_start_nrt_profile.restype = ctypes.c_int64
    lib.axon_stop_nrt_profile.argtypes = [ctypes.c_char_p]
    lib.axon_stop_nrt_profile.restype = ctypes.c_int64

    @contextlib.contextmanager
    def _hook(output_dir, device_ids):
        import jax
        jax.devices()
        if device_ids:
            ids = (ctypes.c_int64 * len(device_ids))(*device_ids)
            rc = lib.axon_start_nrt_profile(ids, len(device_ids))
        else:
            rc = lib.axon_start_nrt_profile(None, 0)
        if rc != 0:
            raise RuntimeError(f"axon_start_nrt_profile rc={rc}")
        try:
            yield
        finally:
            n = lib.axon_stop_nrt_profile(str(output_dir).encode())
            print(f"profile: {n} file(s) written to {output_dir}")

    mod.set_axon_ntff_profile_hook(_hook)


def run(x, weight, bias, dec_w, dec_b, trace=False):
    from concourse import bass_utils
    from concourse.bass_utils import run_bass_kernel_spmd

    if trace:
        _install_ntff_hook()
        # artifact upload needs a bucket that doesn't exist here
        bass_utils.upload_artifacts = lambda tmpdir: tmpdir

    nc = _get_nc()
    in_maps = make_in_maps(x, weight, bias, dec_w)
    r = run_bass_kernel_spmd(nc, in_maps, list(range(NCORES)), trace=trace)
    return combine(r.results, dec_b), r


def kernel(x, weight, bias, dec_w, dec_b):
    out, _ = run(x, weight, bias, dec_w, dec_b, trace=False)
    return out


# revision 33
# speedup vs baseline: 1.1213x; 1.1213x over previous
"""Trainium2 Bass kernel for nn_LCN (locally-connected network) — v2.

Computation:
  x: (512, 1, 280, 280) -> non-overlapping 28x28 patches (10x10 grid, P=100)
  y[b, f, p] = sum_{k,l} x[b, 28ph+k, 28pw+l] * w[f*100+p, 0, k, l]
  y = relu(y + bias[f*100+p]);  out = y_flat @ dec_w.T + dec_b  (j = f*100+p)

Sharding: 8 cores = 2 batch halves (256 imgs) x 4 patch quarters (25 patches).
x is pre-transposed and cast to bf16 on the host into [pt=175, dd=112, b=256]
blocks (pt = patch*7 + chunk, dd = pixel-in-chunk), so the device does ZERO
transposes and half the DMA bytes vs fp32.

Per core pipeline (all matmuls bf16, 1 cycle/row vs fp32's 4):
  - 7 group DMAs (4 patches each; last group 1 patch), ~1.6 MB apiece,
    rearranged in-flight to SBUF [112, (pt b)]
  - per group: 28 accumulating matmuls lhsT=w[112,16], rhs=x[112,256], four
    patches packed per PSUM bank at partition offsets 0/32/64/96
  - ACT: relu(y + bias) -> y_sb bf16
  - decoder matmul accumulates dec[128,10]^T @ y[128,256] -> out_ps [10,256]
Host combines: out[b] = sum over 4 patch-quarters of partial[o,b] + dec_b.
"""

import sys

import numpy as np

for _p in ("/opt/trn_rl_repo", "/opt/trn_rl_repo/concourse"):
    if _p not in sys.path:
        sys.path.insert(0, _p)

import concourse.bass as bass
import concourse.mybir as mybir
import concourse.tile as tile
from concourse import bacc

F32 = mybir.dt.float32
BF16 = mybir.dt.bfloat16

# Problem constants
B, H, W = 512, 280, 280
KS = 28
HS = WS = 10
P = 100
F = 16
OUT = 10
NCORES = 8
NB = 256        # images per core (batch half)
NP = 25         # patches per core (patch quarter)
NT = 7          # 112-pixel chunks per patch
CK = 112        # contraction chunk (4 rows x 28 cols)
NG = 7          # 4-patch groups per core (6 full + 1 single)
XCOLS = NP * NT * NB        # 44800 columns: (p, t, b), b innermost


def build_program():
    nc = bacc.Bacc("TRN2")
    x_d = nc.dram_tensor("x", [NG, CK * 28 * NB], BF16, kind="ExternalInput")
    w_d = nc.dram_tensor("w", [CK, NP * NT * F], BF16, kind="ExternalInput")
    b_d = nc.dram_tensor("bias", [128, NG], F32, kind="ExternalInput")
    d_d = nc.dram_tensor("dec", [128, NG * OUT], BF16, kind="ExternalInput")
    o_d = nc.dram_tensor("out", [OUT, NB], F32, kind="ExternalOutput")

    with tile.TileContext(nc) as tc:
        with (
            tc.tile_pool(name="const", bufs=1) as constp,
            tc.tile_pool(name="xg", bufs=7) as xgp,
            tc.tile_pool(name="yps", bufs=4, space="PSUM") as ypsp,
            tc.tile_pool(name="ops", bufs=1, space="PSUM") as opsp,
        ):
            zero_sb = constp.tile([128, NB], F32)
            nc.vector.memset(zero_sb[:], 0.0)
            # tiny consts head the sync ring (land first); big w on scalar
            bias_sb = constp.tile([128, NG], F32)
            nc.sync.dma_start(out=bias_sb[:], in_=b_d[:])
            dec_sb = constp.tile([128, NG * OUT], BF16)
            nc.sync.dma_start(out=dec_sb[:], in_=d_d[:])
            w_sb = constp.tile([CK, NP * NT * F], BF16)
            nc.scalar.dma_start(out=w_sb[:], in_=w_d[:])
            y_sb = constp.tile([128, NG * NB], BF16)

            # x group DMAs up front, each group split across BOTH HWDGE
            # rings (sync + scalar) at matching positions so both halves
            # drain concurrently and groups still complete in order.
            xg_tiles = []
            for g in range(NG):
                npg = 4 if g < NG - 1 else NP - 4 * (NG - 1)
                t_ = xgp.tile([CK, 28 * NB], BF16, name="xg")
                cols = npg * NT * NB
                hcols = (2 * NT * NB) if npg == 4 else cols
                src = x_d[g:g + 1, :CK * cols].rearrange(
                    "a (dd c) -> (a dd) c", dd=CK)
                nc.sync.dma_start(out=t_[:, :hcols], in_=src[:, :hcols])
                if hcols < cols:
                    nc.scalar.dma_start(
                        out=t_[:, hcols:cols], in_=src[:, hcols:])
                xg_tiles.append(t_)

            def dec_mm(g):
                nc.tensor.matmul(
                    out_ps[:OUT, :NB],
                    dec_sb[:, g * OUT:(g + 1) * OUT],
                    y_sb[:, g * NB:(g + 1) * NB],
                    start=(g == 0),
                    stop=(g == NG - 1),
                )

            out_ps = opsp.tile([128, 512], F32)
            for g in range(NG):
                npg = 4 if g < NG - 1 else NP - 4 * (NG - 1)
                yt = ypsp.tile([128, 512], F32, name="y_ps")
                # make gap partitions (32q+16..32q+32) finite for the ACT read
                nc.vector.tensor_copy(yt[:, :NB], zero_sb[:])
                xg = xg_tiles[g]
                for q in range(npg):
                    pl = 4 * g + q
                    for t in range(NT):
                        nc.tensor.matmul(
                            yt[32 * q:32 * q + F, :NB],
                            w_sb[:, (pl * NT + t) * F:(pl * NT + t + 1) * F],
                            xg[:, (q * NT + t) * NB:(q * NT + t + 1) * NB],
                            start=(t == 0),
                            stop=(t == NT - 1),
                            tile_position=(0, 32 * q),
                        )
                nc.scalar.activation(
                    out=y_sb[:, g * NB:(g + 1) * NB],
                    in_=yt[:, :NB],
                    func=mybir.ActivationFunctionType.Relu,
                    bias=bias_sb[:, g:g + 1],
                )
                # decoder matmul lags one group so its wait on ACT(g-1)
                # never blocks group g's matmuls in the strict-FIFO PE queue
                if g > 0:
                    dec_mm(g - 1)
            dec_mm(NG - 1)

            out_sb = constp.tile([OUT, NB], F32)
            nc.vector.tensor_copy(out_sb[:], out_ps[:OUT, :NB])
            nc.sync.dma_start(out=o_d[:], in_=out_sb[:])

    return nc


def stage_quarter(weight, bias, dec_w, qc):
    """Stage weights/bias/decoder for patch quarter qc (patches 25qc..25qc+24)."""
    import ml_dtypes

    weight = np.asarray(weight, np.float32)
    bias = np.asarray(bias, np.float32)
    dec_w = np.asarray(dec_w, np.float32)

    # w: (1600,1,28,28) -> (f, ph, pw, t, k4, l) -> [dd=(k4 l), (p t f)]
    w6 = weight.reshape(F, HS, WS, NT, 4, KS).transpose(4, 5, 1, 2, 3, 0)
    wt = np.ascontiguousarray(
        w6.reshape(CK, P, NT, F)[:, 25 * qc:25 * qc + NP])
    wst = wt.reshape(CK, NP * NT * F).astype(ml_dtypes.bfloat16)

    b5 = bias.reshape(F, P)
    d5 = dec_w.reshape(OUT, F, P)
    bst = np.zeros((128, NG), np.float32)
    dst = np.zeros((128, NG * OUT), np.float32)
    for pl in range(NP):
        g, q = divmod(pl, 4)
        p = 25 * qc + pl
        bst[32 * q:32 * q + F, g] = b5[:, p]
        dst[32 * q:32 * q + F, g * OUT:(g + 1) * OUT] = d5[:, :, p].T
    return wst, bst, dst.astype(ml_dtypes.bfloat16)


def stage_x(x):
    """x (512,1,280,280) f32 -> global [dd=112, p, t, b] bf16 (u16 view).

    Two cache-friendly steps: permute to (b, p, t, dd) with the 112-byte
    l-runs contiguous, then 700 small L2-resident [512,112]->[112,512]
    block transposes to get dd-major.
    """
    import ml_dtypes

    xr = np.asarray(x, np.float32).reshape(B, HS, NT, 4, WS, KS)
    # (b, ph, t, k4, pw, l) -> (b, ph, pw, t, k4, l)
    y1 = np.ascontiguousarray(xr.transpose(0, 1, 4, 2, 3, 5))
    y1 = y1.reshape(B, P * NT, CK).astype(ml_dtypes.bfloat16).view(np.uint16)
    g = np.empty((CK, P * NT, B), np.uint16)
    for pt in range(P * NT):
        g[:, pt, :] = y1[:, pt, :].T
    return g.reshape(CK, P, NT, B)


_cache = {}


def _get_nc():
    if "nc" not in _cache:
        nc = build_program()
        nc.finalize()
        _cache["nc"] = nc
    return _cache["nc"]


def make_in_maps(x, weight, bias, dec_w):
    import ml_dtypes

    xh = stage_x(x)  # (112, 100, 7, 512) u16 (bf16 bits)
    quarters = [stage_quarter(weight, bias, dec_w, qc) for qc in range(4)]
    in_maps = []
    for core in range(NCORES):
        h, qc = divmod(core, 4)
        xs = np.ascontiguousarray(
            xh[:, 25 * qc:25 * qc + NP, :, NB * h:NB * h + NB]
        ).reshape(CK, XCOLS)
        # chunk-contiguous layout: row g = group g's [dd, cols] block flat
        xr = np.zeros((NG, CK * 28 * NB), np.uint16)
        for g in range(NG):
            npg = 4 if g < NG - 1 else NP - 4 * (NG - 1)
            cols = npg * NT * NB
            c0 = g * 28 * NB
            xr[g, :CK * cols] = xs[:, c0:c0 + cols].reshape(-1)
        wst, bst, dst = quarters[qc]
        in_maps.append({"x": xr.view(ml_dtypes.bfloat16),
                        "w": wst, "bias": bst, "dec": dst})
    return in_maps


def combine(results, dec_b):
    out = np.zeros((B, OUT), np.float32)
    dec_b = np.asarray(dec_b, np.float32)
    for h in range(2):
        acc = np.zeros((OUT, NB), np.float32)
        for qc in range(4):
            acc += results[h * 4 + qc]["out"]
        out[NB * h:NB * h + NB] = acc.T + dec_b
    return out


def _install_ntff_hook():
    """Provide the missing antenv.axon_hooks module so trace=True works
    under axon (replicates trn_boot._ntff_profile_via_ctypes)."""
    import contextlib
    import ctypes
    import types

    if "antenv.axon_hooks" in sys.modules:
        return
    so_path = "/opt/axon/libaxon_pjrt.so"
    holder = {}
    mod = types.ModuleType("antenv.axon_hooks")
    mod.set_axon_ntff_profile_hook = lambda h: holder.__setitem__("h", h)
    mod.get_axon_ntff_profile_hook = lambda: holder.get("h")
    sys.modules["antenv.axon_hooks"] = mod
    try:
        import antenv
        antenv.axon_hooks = mod
    except ImportError:
        pass

    lib = ctypes.CDLL(so_path)
    if not hasattr(lib, "axon_start_nrt_profile"):
        return
    lib.axon_start_nrt_profile.argtypes = [
        ctypes.POINTER(ctypes.c_int64), ctypes.c_size_t]
    lib.axon_start_nrt_profile.restype = ctypes.c_int64
    lib.axon_stop_nrt_profile.argtypes = [ctypes.c_char_p]
    lib.axon_stop_nrt_profile.restype = ctypes.c_int64

    @contextlib.contextmanager
    def _hook(output_dir, device_ids):
        import jax
        jax.devices()
        if device_ids:
            ids = (ctypes.c_int64 * len(device_ids))(*device_ids)
            rc = lib.axon_start_nrt_profile(ids, len(device_ids))
        else:
            rc = lib.axon_start_nrt_profile(None, 0)
        if rc != 0:
            raise RuntimeError(f"axon_start_nrt_profile rc={rc}")
        try:
            yield
        finally:
            n = lib.axon_stop_nrt_profile(str(output_dir).encode())
            print(f"profile: {n} file(s) written to {output_dir}")

    mod.set_axon_ntff_profile_hook(_hook)


def run(x, weight, bias, dec_w, dec_b, trace=False):
    from concourse import bass_utils
    from concourse.bass_utils import run_bass_kernel_spmd

    if trace:
        _install_ntff_hook()
        # artifact upload needs a bucket that doesn't exist here
        bass_utils.upload_artifacts = lambda tmpdir: tmpdir

    nc = _get_nc()
    in_maps = make_in_maps(x, weight, bias, dec_w)
    r = run_bass_kernel_spmd(nc, in_maps, list(range(NCORES)), trace=trace)
    return combine(r.results, dec_b), r


def kernel(x, weight, bias, dec_w, dec_b):
    out, _ = run(x, weight, bias, dec_w, dec_b, trace=False)
    return out
